# revision 2
# baseline (speedup 1.0000x reference)
"""EncoderG v2: Horner-form TAGConv, fp8 DoubleRow hops, fp8 AllGather wire.

Per core (8-way node row-sharding, R=512 local nodes):
  conv1:  y = g0 + A(g1 + A(g2 + A g3)),  g_k = x @ W1_k   (chain width H=256)
  conv2:  z = g'0 + A(g'1 + A(g'2 + A g'3)) + h @ Wm,  g'_k = h @ W2_k (width Z)
g3 is computed for ALL nodes locally (x replicated) -> only 2 AGs in conv1.
Hops run on the PE in feat-form (out [feat,128 x local,512]) as fp8e4
DoubleRow matmuls vs AT (A^T * 2^16, fp8).  Chain values are quantized to
fp8 (x8) by ACT copies, allgathered in feat-form, and re-laid-out to
node-form chain tiles by strided DMA reloads.  Pre-projections g_k run in
bf16 (weights pre-scaled by 2^19 = PSUM units).  BN/ReLU folded into one ACT
op; conv biases folded into BN shift / final bias.  G/L branches are
stagger-interleaved so each branch's AG hides under the other's compute.
"""
import numpy as np

N, D, H, Z, KHOPS = 4096, 512, 256, 128, 3
NCORES = 8
R = N // NCORES          # 512 local nodes
P = 128
KT = N // P              # 32 node blocks
GRP = 4                  # node blocks per chain tile
KG = KT // GRP           # 8 chain tiles
DT1 = D // P             # 4 d-blocks
HT = H // P              # 2 hidden feature tiles
NCH = N // R             # 8 node chunks (g3_full)
EPS = 1e-3

SA = 2.0 ** 16           # A scale
SV = 8.0                 # chain value scale
SX = 4.0                 # x fp8 scale
SW3 = SV / SX            # w1_3 fp8 scale (PSUM = v3 * SV)
UPS = SA * SV            # PSUM units of hop accumulators (2^19)


def build(T=1):
    import concourse.bacc as bacc
    import concourse.tile as tile
    import concourse.mybir as mybir

    F32 = mybir.dt.float32
    BF16 = mybir.dt.bfloat16
    F8 = mybir.dt.float8e4
    AF = mybir.ActivationFunctionType
    DR = mybir.MatmulPerfMode.DoubleRow

    nc = bacc.Bacc("TRN2", target_bir_lowering=False, debug=False,
                   num_devices=NCORES)

    # all inputs are pre-laid-out host-side as [128, X] SBUF images
    at8 = {t: nc.dram_tensor(f"at8_{t}", [P, KG * GRP * R], F8,
                             kind="ExternalInput") for t in "GL"}
    xt8 = nc.dram_tensor("xt8", [P, NCH * DT1 * R], F8, kind="ExternalInput")
    xt16 = nc.dram_tensor("xt16", [P, DT1 * R], BF16, kind="ExternalInput")
    w13 = {t: nc.dram_tensor(f"w13_{t}", [P, HT * DT1 * P], F8,
                             kind="ExternalInput") for t in "GL"}
    w1e = {t: nc.dram_tensor(f"w1e_{t}", [P, 3 * DT1 * H], BF16,
                             kind="ExternalInput") for t in "GL"}
    w2e = {t: nc.dram_tensor(f"w2e_{t}", [P, (KHOPS + 1) * HT * Z], BF16,
                             kind="ExternalInput") for t in "GL"}
    wme = {t: nc.dram_tensor(f"wme_{t}", [P, HT * Z], BF16,
                             kind="ExternalInput") for t in "GL"}
    bnsc = {t: nc.dram_tensor(f"bnsc_{t}", [P, HT], F32,
                              kind="ExternalInput") for t in "GL"}
    bnsh = {t: nc.dram_tensor(f"bnsh_{t}", [P, HT], F32,
                              kind="ExternalInput") for t in "GL"}
    zbias = nc.dram_tensor("zbias", [Z, 1], F32, kind="ExternalInput")
    out_t = nc.dram_tensor("out_t", [Z, R], F32, kind="ExternalOutput")

    RG = [list(range(NCORES))]

    with tile.TileContext(nc) as tc:
        with (
            tc.tile_pool(name="constp", bufs=1) as constp,
            tc.tile_pool(name="wp", bufs=2) as wp,
            tc.tile_pool(name="c1chain", bufs=KG) as c1chain,
            tc.tile_pool(name="c2chain", bufs=KG) as c2chain,
            tc.tile_pool(name="shardp", bufs=4) as shardp,
            tc.tile_pool(name="hp", bufs=2) as hp,
            tc.tile_pool(name="hopG_ps", bufs=3, space="PSUM") as hopG_ps,
            tc.tile_pool(name="hopL_ps", bufs=3, space="PSUM") as hopL_ps,
            tc.tile_pool(name="acc2_ps", bufs=1, space="PSUM") as acc2_ps,
            tc.tile_pool(name="agin", bufs=4, space="DRAM") as agin,
            tc.tile_pool(name="agout", bufs=4, space="DRAM") as agout,
            tc.tile_pool(name="g3dp", bufs=2, space="DRAM") as g3dp,
        ):
            dma_rr = [0]

            def dma(out_ap, in_ap):
                eng = (nc.sync, nc.vector)[dma_rr[0] % 2]
                dma_rr[0] += 1
                eng.dma_start(out_ap, in_ap)

            def pair3(ap2d, lo, n):
                # [P, 2*n] column span -> [P, 2, n] DoubleRow operand
                return ap2d[:, lo:lo + 2 * n].rearrange(
                    "p (i n) -> p i n", i=2)

            for rep in range(T):
                state = {"n2": 0}
                ACC2_TOTAL = 2 * (GRP * KG // 2 + 2 + 2)  # per branch: 16 DR + 4

                def acc2_mm(*args, **kw):
                    kw["start"] = state["n2"] == 0
                    kw["stop"] = state["n2"] == ACC2_TOTAL - 1
                    nc.tensor.matmul(state["acc2"][:], *args, **kw)
                    state["n2"] += 1

                # ---- setup loads ----
                at_t = {}
                for t in "GL":
                    at_t[t] = [wp.tile([P, GRP * R], F8, name=f"at{t}{g}_{rep}",
                                       tag=f"at{t}{g}") for g in range(KG)]
                    for g in range(KG):
                        dma(at_t[t][g][:], at8[t][:, g * GRP * R:(g + 1) * GRP * R])
                xt8_t = constp.tile([P, NCH * DT1 * R], F8, name=f"xt8_{rep}",
                                    tag="xt8")
                dma(xt8_t[:], xt8[:])
                xt16_t = constp.tile([P, DT1 * R], BF16, name=f"xt16_{rep}",
                                     tag="xt16")
                dma(xt16_t[:], xt16[:])
                w_t = {}
                for t in "GL":
                    w_t[t] = {}
                    for nm, dram, sz, dt in (
                        ("w13", w13[t], HT * DT1 * P, F8),
                        ("w1e", w1e[t], 3 * DT1 * H, BF16),
                        ("w2e", w2e[t], (KHOPS + 1) * HT * Z, BF16),
                        ("wme", wme[t], HT * Z, BF16),
                    ):
                        w_t[t][nm] = wp.tile([P, sz], dt, name=f"{nm}{t}_{rep}",
                                             tag=f"{nm}{t}")
                        dma(w_t[t][nm][:], dram[:])
                    for nm, dram in (("bnsc", bnsc[t]), ("bnsh", bnsh[t])):
                        w_t[t][nm] = wp.tile([P, HT], F32, name=f"{nm}{t}_{rep}",
                                             tag=f"{nm}{t}")
                        dma(w_t[t][nm][:], dram[:])
                zb_t = constp.tile([Z, 1], F32, name=f"zb_{rep}", tag="zb")
                dma(zb_t[:], zbias[:])

                state["acc2"] = acc2_ps.tile([Z, R], F32, name=f"acc2_{rep}",
                                             tag="acc2")

                def reload_c1(src_dram, tag, src_is_g3):
                    """strided reload -> conv1 chain tiles [p, hf,4b,128]."""
                    tiles = []
                    for j in range(KG):
                        ct = c1chain.tile([P, HT * GRP * P], F8,
                                          name=f"c1_{tag}_{j}", tag=f"c1{tag[-1]}")
                        for hf in range(HT):
                            if src_is_g3:
                                # g3d [H, N]: addr(f, n) = f*N + n
                                src = src_dram[hf * P:(hf + 1) * P,
                                               j * R:(j + 1) * R].rearrange(
                                    "w (b p) -> p b w", p=P)
                            else:
                                # bounce [8*H, R]: rows c*H + hf*128 + w
                                src = src_dram[j * H + hf * P:
                                               j * H + (hf + 1) * P, :].rearrange(
                                    "w (b p) -> p b w", p=P)
                            dma(ct[:, hf * GRP * P:(hf + 1) * GRP * P].rearrange(
                                "p (b w) -> p b w", b=GRP), src)
                        tiles.append(ct)
                    return tiles

                def reload_c2(src_dram, tag):
                    tiles = []
                    for j in range(KG):
                        ct = c2chain.tile([P, GRP * P], F8,
                                          name=f"c2_{tag}_{j}", tag=f"c2{tag[-1]}")
                        src = src_dram[j * Z:(j + 1) * Z, :].rearrange(
                            "w (b p) -> p b w", p=P)
                        dma(ct[:].rearrange("p (b w) -> p b w", b=GRP), src)
                        tiles.append(ct)
                    return tiles

                def hop_c1(psums, chain, at, w1et, kblk):
                    """PSUM[hf] = g_kblk (bf16) + A-hop (fp8 DR) over chain."""
                    for hf in range(HT):
                        for db in range(DT1):
                            lhsT = w1et[:, ((kblk * DT1 + db) * H + hf * P):
                                        ((kblk * DT1 + db) * H + hf * P) + P]
                            nc.tensor.matmul(
                                psums[hf][:], lhsT,
                                xt16_t[:, db * R:(db + 1) * R],
                                start=(db == 0), stop=False)
                    for hf in range(HT):
                        for g in range(KT // 2):
                            ct = chain[g // 2]
                            lo = hf * GRP * P + (g % 2) * 2 * P
                            nc.tensor.matmul(
                                psums[hf][:], pair3(ct[:], lo, P),
                                pair3(at[g // 2][:], (g % 2) * 2 * R, R),
                                start=False, stop=(g == KT // 2 - 1),
                                perf_mode=DR)

                def quant_c1(psums, tag):
                    """PSUM -> fp8 shard [2*128, 512] in DRAM bounce-in."""
                    bi = agin.tile([H, R], F8, name=f"bi_{tag}", tag="agin1")
                    for hf in range(HT):
                        sh = shardp.tile([P, R], F8, name=f"sh_{tag}_{hf}",
                                         tag="shard")
                        nc.scalar.activation(sh[:], psums[hf][:], AF.Copy,
                                             scale=1.0 / SA)
                        dma(bi[hf * P:(hf + 1) * P, :], sh[:])
                    return bi

                def allgather(bi, rows, tag):
                    bo = agout.tile([NCORES * rows, R], F8, name=f"bo_{tag}",
                                    tag="agout", addr_space="Shared")
                    nc.gpsimd.collective_compute(
                        "AllGather", mybir.AluOpType.bypass, replica_groups=RG,
                        ins=[bi.opt()], outs=[bo.opt()])
                    return bo

                def branch(t):
                    at = at_t[t]
                    w = w_t[t]

                    # ---- g3_full: feat-form fp8-DR, DRAM round trip ----
                    g3d = g3dp.tile([H, N], F8, name=f"g3d{t}_{rep}", tag="g3d")
                    for ch in range(NCH):
                        for hf in range(HT):
                            ps = hop_ps.tile([P, R], F32,
                                             name=f"g3p{t}{ch}_{hf}",
                                             tag=f"hop{t}")
                            for dp in range(DT1 // 2):
                                nc.tensor.matmul(
                                    ps[:],
                                    pair3(w["w13"][:],
                                          hf * DT1 * P + dp * 2 * P, P),
                                    pair3(xt8_t[:],
                                          ch * DT1 * R + dp * 2 * R, R),
                                    start=(dp == 0), stop=(dp == DT1 // 2 - 1),
                                    perf_mode=DR)
                            q = shardp.tile([P, R], F8, name=f"g3q{t}{ch}_{hf}",
                                            tag="shard")
                            nc.scalar.activation(q[:], ps[:], AF.Copy)
                            dma(g3d[hf * P:(hf + 1) * P, ch * R:(ch + 1) * R],
                                q[:])
                        if ch % 4 == 3:
                            yield
                    chain = reload_c1(g3d, f"g3{t}{rep}", True)
                    yield

                    # ---- conv1 hops: v2, v1 ----
                    for kblk, nm in ((2, "v2"), (1, "v1")):
                        psums = [hop_ps.tile([P, R], F32,
                                             name=f"{nm}{t}p{hf}", tag=f"hop{t}")
                                 for hf in range(HT)]
                        hop_c1(psums, chain, at, w["w1e"], kblk)
                        bi = quant_c1(psums, f"{nm}{t}{rep}")
                        bo = allgather(bi, H, f"{nm}{t}{rep}")
                        yield
                        chain = reload_c1(bo, f"{nm}{t}{rep}", False)
                        yield

                    # ---- y + BN/ReLU -> h ----
                    psums = [hop_ps.tile([P, R], F32, name=f"y{t}p{hf}",
                                         tag=f"hop{t}") for hf in range(HT)]
                    hop_c1(psums, chain, at, w["w1e"], 0)
                    h_t = hp.tile([P, HT * R], BF16, name=f"h{t}_{rep}",
                                  tag=f"h{t}")
                    for hf in range(HT):
                        nc.scalar.activation(h_t[:, hf * R:(hf + 1) * R],
                                             psums[hf][:], AF.Relu,
                                             bias=w["bnsh"][:, hf:hf + 1],
                                             scale=w["bnsc"][:, hf:hf + 1])
                    yield

                    # ---- conv2: v'3 ----
                    def w2s(k):
                        return [w["w2e"][:, (k * HT + hb) * Z:
                                         (k * HT + hb) * Z + Z]
                                for hb in range(HT)]

                    ps = hop_ps.tile([Z, R], F32, name=f"vp3{t}", tag=f"hop{t}")
                    for hb in range(HT):
                        nc.tensor.matmul(ps[:], w2s(3)[hb],
                                         h_t[:, hb * R:(hb + 1) * R],
                                         start=(hb == 0), stop=(hb == HT - 1))
                    bi = agin.tile([Z, R], F8, name=f"bi_vp3{t}{rep}",
                                   tag="agin2")
                    sh = shardp.tile([Z, R], F8, name=f"sh_vp3{t}", tag="shard")
                    nc.scalar.activation(sh[:], ps[:], AF.Copy)
                    dma(bi[:], sh[:])
                    bo = allgather(bi, Z, f"vp3{t}{rep}")
                    yield
                    chain2 = reload_c2(bo, f"vp3{t}{rep}")
                    yield

                    # ---- conv2 hops: v'2, v'1 ----
                    for k, nm in ((2, "vp2"), (1, "vp1")):
                        ps = hop_ps.tile([Z, R], F32, name=f"{nm}{t}",
                                         tag=f"hop{t}")
                        for hb in range(HT):
                            nc.tensor.matmul(ps[:], w2s(k)[hb],
                                             h_t[:, hb * R:(hb + 1) * R],
                                             start=(hb == 0), stop=False)
                        for g in range(KT // 2):
                            nc.tensor.matmul(
                                ps[:], pair3(chain2[g // 2][:], (g % 2) * 2 * P, P),
                                pair3(at[g // 2][:], (g % 2) * 2 * R, R),
                                start=False, stop=(g == KT // 2 - 1),
                                perf_mode=DR)
                        bi = agin.tile([Z, R], F8, name=f"bi_{nm}{t}{rep}",
                                       tag="agin2")
                        sh = shardp.tile([Z, R], F8, name=f"sh_{nm}{t}",
                                         tag="shard")
                        nc.scalar.activation(sh[:], ps[:], AF.Copy,
                                             scale=1.0 / SA)
                        dma(bi[:], sh[:])
                        bo = allgather(bi, Z, f"{nm}{t}{rep}")
                        yield
                        chain2 = reload_c2(bo, f"{nm}{t}{rep}")
                        yield

                    # ---- z accumulation (shared acc2) ----
                    for hb in range(HT):
                        acc2_mm(w2s(0)[hb], h_t[:, hb * R:(hb + 1) * R])
                    for hb in range(HT):
                        acc2_mm(w["wme"][:, hb * Z:(hb + 1) * Z],
                                h_t[:, hb * R:(hb + 1) * R])
                    for g in range(KT // 2):
                        acc2_mm(pair3(chain2[g // 2][:], (g % 2) * 2 * P, P),
                                pair3(at[g // 2][:], (g % 2) * 2 * R, R),
                                perf_mode=DR)

                gens = [branch("G"), branch("L")]
                done = [False, False]
                while not all(done):
                    for i, g in enumerate(gens):
                        if not done[i]:
                            try:
                                next(g)
                            except StopIteration:
                                done[i] = True

                out_sb = shardp.tile([Z, R], F32, name=f"out_{rep}", tag="outs")
                nc.scalar.activation(out_sb[:], state["acc2"][:], AF.Copy,
                                     bias=zb_t[:, 0:1], scale=1.0 / UPS)
                dma(out_t[:], out_sb[:])

    nc.compile()
    return nc


def make_in_maps(inputs):
    import ml_dtypes
    bf16 = ml_dtypes.bfloat16
    f8 = ml_dtypes.float8_e4m3

    x = np.asarray(inputs["x"], np.float64)
    xt = x.T  # [D, N]
    # xt8: [p, chunk, db, n] image
    xt8 = (xt * SX).reshape(DT1, P, NCH, R).transpose(1, 2, 0, 3).reshape(
        P, NCH * DT1 * R).astype(f8)
    prep = {"xt8": np.ascontiguousarray(xt8)}
    for t in "GL":
        A = np.asarray(inputs[f"A_{t}"], np.float64)
        # at8 image: [p, (g b) r] for local cols; per-core below
        prep[f"_at8_{t}"] = (A.T * SA).astype(f8)  # [N, N] cols=rows of A
        W1 = np.asarray(inputs[f"W1_{t}"], np.float64)
        W2 = np.asarray(inputs[f"W2_{t}"], np.float64)
        Wm = np.asarray(inputs[f"Wm_{t}"], np.float64)
        w13 = (W1[3 * D:] * SW3).reshape(DT1, P, HT, P).transpose(
            1, 2, 0, 3).reshape(P, HT * DT1 * P).astype(f8)
        prep[f"w13_{t}"] = np.ascontiguousarray(w13)
        w1e = (W1[:3 * D] * UPS).reshape(3, DT1, P, H).transpose(
            2, 0, 1, 3).reshape(P, 3 * DT1 * H).astype(bf16)
        prep[f"w1e_{t}"] = np.ascontiguousarray(w1e)
        w2e = np.concatenate([
            W2[:H] * UPS, W2[H:2 * H] * UPS, W2[2 * H:3 * H] * UPS,
            W2[3 * H:] * SV]).reshape(KHOPS + 1, HT, P, Z).transpose(
            2, 0, 1, 3).reshape(P, (KHOPS + 1) * HT * Z).astype(bf16)
        prep[f"w2e_{t}"] = np.ascontiguousarray(w2e)
        wme = (Wm * UPS).reshape(HT, P, Z).transpose(1, 0, 2).reshape(
            P, HT * Z).astype(bf16)
        prep[f"wme_{t}"] = np.ascontiguousarray(wme)
        g = np.asarray(inputs[f"gamma_{t}"], np.float64)
        b = np.asarray(inputs[f"beta_{t}"], np.float64)
        mu = np.asarray(inputs[f"mean_{t}"], np.float64)
        v = np.asarray(inputs[f"var_{t}"], np.float64)
        b1 = np.asarray(inputs[f"b1_{t}"], np.float64)
        sc = g / np.sqrt(v + EPS)
        sh = (b1 - mu) * sc + b
        prep[f"bnsc_{t}"] = np.ascontiguousarray(
            (sc / UPS).reshape(HT, P).T.astype(np.float32))
        prep[f"bnsh_{t}"] = np.ascontiguousarray(
            sh.reshape(HT, P).T.astype(np.float32))
    zb = sum(np.asarray(inputs[f"b2_{t}"], np.float64) +
             np.asarray(inputs[f"bm_{t}"], np.float64) for t in "GL")
    prep["zbias"] = np.ascontiguousarray(zb.reshape(Z, 1).astype(np.float32))

    in_maps = []
    for c in range(NCORES):
        sl = slice(c * R, (c + 1) * R)
        m = {k: v for k, v in prep.items() if not k.startswith("_")}
        m["xt16"] = np.ascontiguousarray(
            x.T[:, sl].reshape(DT1, P, R).transpose(1, 0, 2).reshape(
                P, DT1 * R).astype(bf16))
        for t in "GL":
            atl = prep[f"_at8_{t}"][:, sl]  # [N, R]
            m[f"at8_{t}"] = np.ascontiguousarray(
                atl.reshape(KG, GRP, P, R).transpose(2, 0, 1, 3).reshape(
                    P, KG * GRP * R))
        in_maps.append(m)
    return in_maps


def assemble(results):
    out = np.empty((N, Z), np.float32)
    for c in range(NCORES):
        out[c * R:(c + 1) * R, :] = results[c]["out_t"].T
    return out


_CACHE = {}


def _get_nc():
    if "nc" not in _CACHE:
        _CACHE["nc"] = build(T=1)
    return _CACHE["nc"]


def kernel(**inputs) -> np.ndarray:
    from concourse.bass_utils import run_bass_kernel_spmd

    nc = _get_nc()
    in_maps = make_in_maps(inputs)
    res = run_bass_kernel_spmd(nc, in_maps, list(range(NCORES)))
    return assemble(res.results)


# revision 3
# speedup vs baseline: 1.1176x; 1.1176x over previous
"""EncoderG v2: Horner-form TAGConv, fp8 DoubleRow hops, fp8 AllGather wire.

Per core (8-way node row-sharding, R=512 local nodes):
  conv1:  y = g0 + A(g1 + A(g2 + A g3)),  g_k = x @ W1_k   (chain width H=256)
  conv2:  z = g'0 + A(g'1 + A(g'2 + A g'3)) + h @ Wm,  g'_k = h @ W2_k (width Z)
g3 is computed for ALL nodes locally (x replicated) -> only 2 AGs in conv1.
Hops run on the PE in feat-form (out [feat,128 x local,512]) as fp8e4
DoubleRow matmuls vs AT (A^T * 2^16, fp8).  Chain values are quantized to
fp8 (x8) by ACT copies, allgathered in feat-form, and re-laid-out to
node-form chain tiles by strided DMA reloads.  Pre-projections g_k run in
bf16 (weights pre-scaled by 2^19 = PSUM units).  BN/ReLU folded into one ACT
op; conv biases folded into BN shift / final bias.  G/L branches are
stagger-interleaved so each branch's AG hides under the other's compute.
"""
import numpy as np

N, D, H, Z, KHOPS = 4096, 512, 256, 128, 3
NCORES = 8
R = N // NCORES          # 512 local nodes
P = 128
KT = N // P              # 32 node blocks
GRP = 4                  # node blocks per chain tile
KG = KT // GRP           # 8 chain tiles
DT1 = D // P             # 4 d-blocks
HT = H // P              # 2 hidden feature tiles
NCH = N // R             # 8 node chunks (g3_full)
EPS = 1e-3

SA = 2.0 ** 16           # A scale
SV = 8.0                 # chain value scale
SX = 4.0                 # x fp8 scale
SW3 = SV / SX            # w1_3 fp8 scale (PSUM = v3 * SV)
UPS = SA * SV            # PSUM units of hop accumulators (2^19)


def build(T=1):
    import concourse.bacc as bacc
    import concourse.tile as tile
    import concourse.mybir as mybir

    F32 = mybir.dt.float32
    BF16 = mybir.dt.bfloat16
    F8 = mybir.dt.float8e4
    AF = mybir.ActivationFunctionType
    DR = mybir.MatmulPerfMode.DoubleRow

    nc = bacc.Bacc("TRN2", target_bir_lowering=False, debug=False,
                   num_devices=NCORES)

    # all inputs are pre-laid-out host-side as [128, X] SBUF images
    at8 = {t: nc.dram_tensor(f"at8_{t}", [P, KG * GRP * R], F8,
                             kind="ExternalInput") for t in "GL"}
    xt8 = nc.dram_tensor("xt8", [P, NCH * DT1 * R], F8, kind="ExternalInput")
    xt16 = nc.dram_tensor("xt16", [P, DT1 * R], BF16, kind="ExternalInput")
    w13 = {t: nc.dram_tensor(f"w13_{t}", [P, HT * DT1 * P], F8,
                             kind="ExternalInput") for t in "GL"}
    w1e = {t: nc.dram_tensor(f"w1e_{t}", [P, 3 * DT1 * H], BF16,
                             kind="ExternalInput") for t in "GL"}
    w2e = {t: nc.dram_tensor(f"w2e_{t}", [P, (KHOPS + 1) * HT * Z], BF16,
                             kind="ExternalInput") for t in "GL"}
    wme = {t: nc.dram_tensor(f"wme_{t}", [P, HT * Z], BF16,
                             kind="ExternalInput") for t in "GL"}
    bnsc = {t: nc.dram_tensor(f"bnsc_{t}", [P, HT], F32,
                              kind="ExternalInput") for t in "GL"}
    bnsh = {t: nc.dram_tensor(f"bnsh_{t}", [P, HT], F32,
                              kind="ExternalInput") for t in "GL"}
    zbias = nc.dram_tensor("zbias", [Z, 1], F32, kind="ExternalInput")
    out_t = nc.dram_tensor("out_t", [Z, R], F32, kind="ExternalOutput")

    RG = [list(range(NCORES))]

    with tile.TileContext(nc) as tc:
        with (
            tc.tile_pool(name="constp", bufs=1) as constp,
            tc.tile_pool(name="wp", bufs=2) as wp,
            tc.tile_pool(name="c1chain", bufs=KG) as c1chain,
            tc.tile_pool(name="c2chain", bufs=KG) as c2chain,
            tc.tile_pool(name="shardp", bufs=4) as shardp,
            tc.tile_pool(name="hp", bufs=2) as hp,
            tc.tile_pool(name="hopG_ps", bufs=2, space="PSUM") as hopG_ps,
            tc.tile_pool(name="hopL_ps", bufs=2, space="PSUM") as hopL_ps,
            tc.tile_pool(name="acc2_ps", bufs=1, space="PSUM") as acc2_ps,
            tc.tile_pool(name="agin", bufs=4, space="DRAM") as agin,
            tc.tile_pool(name="agout", bufs=4, space="DRAM") as agout,
            tc.tile_pool(name="g3dp", bufs=2, space="DRAM") as g3dp,
        ):
            dma_rr = [0]

            def dma(out_ap, in_ap):
                eng = (nc.sync, nc.vector)[dma_rr[0] % 2]
                dma_rr[0] += 1
                eng.dma_start(out_ap, in_ap)

            def pair3(ap2d, lo, n):
                # [P, 2*n] column span -> [P, 2, n] DoubleRow operand
                return ap2d[:, lo:lo + 2 * n].rearrange(
                    "p (i n) -> p i n", i=2)

            for rep in range(T):
                state = {"n2": 0}
                ACC2_TOTAL = 2 * (GRP * KG // 2 + 2 + 2)  # per branch: 16 DR + 4

                def acc2_mm(*args, **kw):
                    kw["start"] = state["n2"] == 0
                    kw["stop"] = state["n2"] == ACC2_TOTAL - 1
                    nc.tensor.matmul(state["acc2"][:], *args, **kw)
                    state["n2"] += 1

                # ---- setup loads ----
                at_t = {}
                for t in "GL":
                    at_t[t] = [wp.tile([P, GRP * R], F8, name=f"at{t}{g}_{rep}",
                                       tag=f"at{t}{g}") for g in range(KG)]
                    for g in range(KG):
                        dma(at_t[t][g][:], at8[t][:, g * GRP * R:(g + 1) * GRP * R])
                xt8_t = constp.tile([P, NCH * DT1 * R], F8, name=f"xt8_{rep}",
                                    tag="xt8")
                dma(xt8_t[:], xt8[:])
                xt16_t = constp.tile([P, DT1 * R], BF16, name=f"xt16_{rep}",
                                     tag="xt16")
                dma(xt16_t[:], xt16[:])
                w_t = {}
                for t in "GL":
                    w_t[t] = {}
                    for nm, dram, sz, dt in (
                        ("w13", w13[t], HT * DT1 * P, F8),
                        ("w1e", w1e[t], 3 * DT1 * H, BF16),
                        ("w2e", w2e[t], (KHOPS + 1) * HT * Z, BF16),
                        ("wme", wme[t], HT * Z, BF16),
                    ):
                        w_t[t][nm] = wp.tile([P, sz], dt, name=f"{nm}{t}_{rep}",
                                             tag=f"{nm}{t}")
                        dma(w_t[t][nm][:], dram[:])
                    for nm, dram in (("bnsc", bnsc[t]), ("bnsh", bnsh[t])):
                        w_t[t][nm] = wp.tile([P, HT], F32, name=f"{nm}{t}_{rep}",
                                             tag=f"{nm}{t}")
                        dma(w_t[t][nm][:], dram[:])
                zb_t = constp.tile([Z, 1], F32, name=f"zb_{rep}", tag="zb")
                dma(zb_t[:], zbias[:])

                state["acc2"] = acc2_ps.tile([Z, R], F32, name=f"acc2_{rep}",
                                             tag="acc2")

                def reload_c1(src_dram, tag, src_is_g3):
                    """strided reload -> conv1 chain tiles [p, hf,4b,128]."""
                    tiles = []
                    for j in range(KG):
                        ct = c1chain.tile([P, HT * GRP * P], F8,
                                          name=f"c1_{tag}_{j}", tag=f"c1{tag[-1]}")
                        for hf in range(HT):
                            if src_is_g3:
                                # g3d [H, N]: addr(f, n) = f*N + n
                                src = src_dram[hf * P:(hf + 1) * P,
                                               j * R:(j + 1) * R].rearrange(
                                    "w (b p) -> p b w", p=P)
                            else:
                                # bounce [8*H, R]: rows c*H + hf*128 + w
                                src = src_dram[j * H + hf * P:
                                               j * H + (hf + 1) * P, :].rearrange(
                                    "w (b p) -> p b w", p=P)
                            dma(ct[:, hf * GRP * P:(hf + 1) * GRP * P].rearrange(
                                "p (b w) -> p b w", b=GRP), src)
                        tiles.append(ct)
                    return tiles

                def reload_c2(src_dram, tag):
                    tiles = []
                    for j in range(KG):
                        ct = c2chain.tile([P, GRP * P], F8,
                                          name=f"c2_{tag}_{j}", tag=f"c2{tag[-1]}")
                        src = src_dram[j * Z:(j + 1) * Z, :].rearrange(
                            "w (b p) -> p b w", p=P)
                        dma(ct[:].rearrange("p (b w) -> p b w", b=GRP), src)
                        tiles.append(ct)
                    return tiles

                def hop_c1(psums, chain, at, w1et, kblk):
                    """PSUM[hf] = g_kblk (bf16) + A-hop (fp8 DR) over chain."""
                    for hf in range(HT):
                        for db in range(DT1):
                            lhsT = w1et[:, ((kblk * DT1 + db) * H + hf * P):
                                        ((kblk * DT1 + db) * H + hf * P) + P]
                            nc.tensor.matmul(
                                psums[hf][:], lhsT,
                                xt16_t[:, db * R:(db + 1) * R],
                                start=(db == 0), stop=False)
                    for hf in range(HT):
                        for g in range(KT // 2):
                            ct = chain[g // 2]
                            lo = hf * GRP * P + (g % 2) * 2 * P
                            nc.tensor.matmul(
                                psums[hf][:], pair3(ct[:], lo, P),
                                pair3(at[g // 2][:], (g % 2) * 2 * R, R),
                                start=False, stop=(g == KT // 2 - 1),
                                perf_mode=DR)

                def quant_c1(psums, tag):
                    """PSUM -> fp8 shard [2*128, 512] in DRAM bounce-in."""
                    bi = agin.tile([H, R], F8, name=f"bi_{tag}", tag="agin1")
                    for hf in range(HT):
                        sh = shardp.tile([P, R], F8, name=f"sh_{tag}_{hf}",
                                         tag="shard")
                        nc.scalar.activation(sh[:], psums[hf][:], AF.Copy,
                                             scale=1.0 / SA)
                        dma(bi[hf * P:(hf + 1) * P, :], sh[:])
                    return bi

                def allgather(bi, rows, tag):
                    bo = agout.tile([NCORES * rows, R], F8, name=f"bo_{tag}",
                                    tag="agout", addr_space="Shared")
                    nc.gpsimd.collective_compute(
                        "AllGather", mybir.AluOpType.bypass, replica_groups=RG,
                        ins=[bi.opt()], outs=[bo.opt()])
                    return bo

                def branch(t):
                    at = at_t[t]
                    w = w_t[t]

                    # ---- g3_full: feat-form fp8-DR, DRAM round trip ----
                    g3d = g3dp.tile([H, N], F8, name=f"g3d{t}_{rep}", tag="g3d")
                    for ch in range(NCH):
                        for hf in range(HT):
                            ps = hop_ps.tile([P, R], F32,
                                             name=f"g3p{t}{ch}_{hf}",
                                             tag=f"hop{t}")
                            for dp in range(DT1 // 2):
                                nc.tensor.matmul(
                                    ps[:],
                                    pair3(w["w13"][:],
                                          hf * DT1 * P + dp * 2 * P, P),
                                    pair3(xt8_t[:],
                                          ch * DT1 * R + dp * 2 * R, R),
                                    start=(dp == 0), stop=(dp == DT1 // 2 - 1),
                                    perf_mode=DR)
                            q = shardp.tile([P, R], F8, name=f"g3q{t}{ch}_{hf}",
                                            tag="shard")
                            nc.scalar.activation(q[:], ps[:], AF.Copy)
                            dma(g3d[hf * P:(hf + 1) * P, ch * R:(ch + 1) * R],
                                q[:])
                        if ch % 4 == 3:
                            yield
                    chain = reload_c1(g3d, f"g3{t}{rep}", True)
                    yield

                    # ---- conv1 hops: v2, v1 ----
                    for kblk, nm in ((2, "v2"), (1, "v1")):
                        psums = [hop_ps.tile([P, R], F32,
                                             name=f"{nm}{t}p{hf}", tag=f"hop{t}")
                                 for hf in range(HT)]
                        hop_c1(psums, chain, at, w["w1e"], kblk)
                        bi = quant_c1(psums, f"{nm}{t}{rep}")
                        bo = allgather(bi, H, f"{nm}{t}{rep}")
                        yield
                        chain = reload_c1(bo, f"{nm}{t}{rep}", False)
                        yield

                    # ---- y + BN/ReLU -> h ----
                    psums = [hop_ps.tile([P, R], F32, name=f"y{t}p{hf}",
                                         tag=f"hop{t}") for hf in range(HT)]
                    hop_c1(psums, chain, at, w["w1e"], 0)
                    h_t = hp.tile([P, HT * R], BF16, name=f"h{t}_{rep}",
                                  tag=f"h{t}")
                    for hf in range(HT):
                        nc.scalar.activation(h_t[:, hf * R:(hf + 1) * R],
                                             psums[hf][:], AF.Relu,
                                             bias=w["bnsh"][:, hf:hf + 1],
                                             scale=w["bnsc"][:, hf:hf + 1])
                    yield

                    # ---- conv2: v'3 ----
                    def w2s(k):
                        return [w["w2e"][:, (k * HT + hb) * Z:
                                         (k * HT + hb) * Z + Z]
                                for hb in range(HT)]

                    ps = hop_ps.tile([Z, R], F32, name=f"vp3{t}", tag=f"hop{t}")
                    for hb in range(HT):
                        nc.tensor.matmul(ps[:], w2s(3)[hb],
                                         h_t[:, hb * R:(hb + 1) * R],
                                         start=(hb == 0), stop=(hb == HT - 1))
                    bi = agin.tile([Z, R], F8, name=f"bi_vp3{t}{rep}",
                                   tag="agin2")
                    sh = shardp.tile([Z, R], F8, name=f"sh_vp3{t}", tag="shard")
                    nc.scalar.activation(sh[:], ps[:], AF.Copy)
                    dma(bi[:], sh[:])
                    bo = allgather(bi, Z, f"vp3{t}{rep}")
                    yield
                    chain2 = reload_c2(bo, f"vp3{t}{rep}")
                    yield

                    # ---- conv2 hops: v'2, v'1 ----
                    for k, nm in ((2, "vp2"), (1, "vp1")):
                        ps = hop_ps.tile([Z, R], F32, name=f"{nm}{t}",
                                         tag=f"hop{t}")
                        for hb in range(HT):
                            nc.tensor.matmul(ps[:], w2s(k)[hb],
                                             h_t[:, hb * R:(hb + 1) * R],
                                             start=(hb == 0), stop=False)
                        for g in range(KT // 2):
                            nc.tensor.matmul(
                                ps[:], pair3(chain2[g // 2][:], (g % 2) * 2 * P, P),
                                pair3(at[g // 2][:], (g % 2) * 2 * R, R),
                                start=False, stop=(g == KT // 2 - 1),
                                perf_mode=DR)
                        bi = agin.tile([Z, R], F8, name=f"bi_{nm}{t}{rep}",
                                       tag="agin2")
                        sh = shardp.tile([Z, R], F8, name=f"sh_{nm}{t}",
                                         tag="shard")
                        nc.scalar.activation(sh[:], ps[:], AF.Copy,
                                             scale=1.0 / SA)
                        dma(bi[:], sh[:])
                        bo = allgather(bi, Z, f"{nm}{t}{rep}")
                        yield
                        chain2 = reload_c2(bo, f"{nm}{t}{rep}")
                        yield

                    # ---- z accumulation (shared acc2) ----
                    for hb in range(HT):
                        acc2_mm(w2s(0)[hb], h_t[:, hb * R:(hb + 1) * R])
                    for hb in range(HT):
                        acc2_mm(w["wme"][:, hb * Z:(hb + 1) * Z],
                                h_t[:, hb * R:(hb + 1) * R])
                    for g in range(KT // 2):
                        acc2_mm(pair3(chain2[g // 2][:], (g % 2) * 2 * P, P),
                                pair3(at[g // 2][:], (g % 2) * 2 * R, R),
                                perf_mode=DR)

                gens = [branch("G"), branch("L")]
                done = [False, False]
                while not all(done):
                    for i, g in enumerate(gens):
                        if not done[i]:
                            try:
                                next(g)
                            except StopIteration:
                                done[i] = True

                out_sb = shardp.tile([Z, R], F32, name=f"out_{rep}", tag="outs")
                nc.scalar.activation(out_sb[:], state["acc2"][:], AF.Copy,
                                     bias=zb_t[:, 0:1], scale=1.0 / UPS)
                dma(out_t[:], out_sb[:])

    nc.compile()
    return nc


def make_in_maps(inputs):
    import ml_dtypes
    bf16 = ml_dtypes.bfloat16
    f8 = ml_dtypes.float8_e4m3

    x = np.asarray(inputs["x"], np.float64)
    xt = x.T  # [D, N]
    # xt8: [p, chunk, db, n] image
    xt8 = (xt * SX).reshape(DT1, P, NCH, R).transpose(1, 2, 0, 3).reshape(
        P, NCH * DT1 * R).astype(f8)
    prep = {"xt8": np.ascontiguousarray(xt8)}
    for t in "GL":
        A = np.asarray(inputs[f"A_{t}"], np.float64)
        # at8 image: [p, (g b) r] for local cols; per-core below
        prep[f"_at8_{t}"] = (A.T * SA).astype(f8)  # [N, N] cols=rows of A
        W1 = np.asarray(inputs[f"W1_{t}"], np.float64)
        W2 = np.asarray(inputs[f"W2_{t}"], np.float64)
        Wm = np.asarray(inputs[f"Wm_{t}"], np.float64)
        w13 = (W1[3 * D:] * SW3).reshape(DT1, P, HT, P).transpose(
            1, 2, 0, 3).reshape(P, HT * DT1 * P).astype(f8)
        prep[f"w13_{t}"] = np.ascontiguousarray(w13)
        w1e = (W1[:3 * D] * UPS).reshape(3, DT1, P, H).transpose(
            2, 0, 1, 3).reshape(P, 3 * DT1 * H).astype(bf16)
        prep[f"w1e_{t}"] = np.ascontiguousarray(w1e)
        w2e = np.concatenate([
            W2[:H] * UPS, W2[H:2 * H] * UPS, W2[2 * H:3 * H] * UPS,
            W2[3 * H:] * SV]).reshape(KHOPS + 1, HT, P, Z).transpose(
            2, 0, 1, 3).reshape(P, (KHOPS + 1) * HT * Z).astype(bf16)
        prep[f"w2e_{t}"] = np.ascontiguousarray(w2e)
        wme = (Wm * UPS).reshape(HT, P, Z).transpose(1, 0, 2).reshape(
            P, HT * Z).astype(bf16)
        prep[f"wme_{t}"] = np.ascontiguousarray(wme)
        g = np.asarray(inputs[f"gamma_{t}"], np.float64)
        b = np.asarray(inputs[f"beta_{t}"], np.float64)
        mu = np.asarray(inputs[f"mean_{t}"], np.float64)
        v = np.asarray(inputs[f"var_{t}"], np.float64)
        b1 = np.asarray(inputs[f"b1_{t}"], np.float64)
        sc = g / np.sqrt(v + EPS)
        sh = (b1 - mu) * sc + b
        prep[f"bnsc_{t}"] = np.ascontiguousarray(
            (sc / UPS).reshape(HT, P).T.astype(np.float32))
        prep[f"bnsh_{t}"] = np.ascontiguousarray(
            sh.reshape(HT, P).T.astype(np.float32))
    zb = sum(np.asarray(inputs[f"b2_{t}"], np.float64) +
             np.asarray(inputs[f"bm_{t}"], np.float64) for t in "GL")
    prep["zbias"] = np.ascontiguousarray(zb.reshape(Z, 1).astype(np.float32))

    in_maps = []
    for c in range(NCORES):
        sl = slice(c * R, (c + 1) * R)
        m = {k: v for k, v in prep.items() if not k.startswith("_")}
        m["xt16"] = np.ascontiguousarray(
            x.T[:, sl].reshape(DT1, P, R).transpose(1, 0, 2).reshape(
                P, DT1 * R).astype(bf16))
        for t in "GL":
            atl = prep[f"_at8_{t}"][:, sl]  # [N, R]
            m[f"at8_{t}"] = np.ascontiguousarray(
                atl.reshape(KG, GRP, P, R).transpose(2, 0, 1, 3).reshape(
                    P, KG * GRP * R))
        in_maps.append(m)
    return in_maps


def assemble(results):
    out = np.empty((N, Z), np.float32)
    for c in range(NCORES):
        out[c * R:(c + 1) * R, :] = results[c]["out_t"].T
    return out


_CACHE = {}


def _get_nc():
    if "nc" not in _CACHE:
        _CACHE["nc"] = build(T=1)
    return _CACHE["nc"]


def kernel(**inputs) -> np.ndarray:
    from concourse.bass_utils import run_bass_kernel_spmd

    nc = _get_nc()
    in_maps = make_in_maps(inputs)
    res = run_bass_kernel_spmd(nc, in_maps, list(range(NCORES)))
    return assemble(res.results)
